# revision 6
# baseline (speedup 1.0000x reference)
"""GIN autoencoder forward pass on 8 Trainium2 NeuronCores.

Strategy (data-parallel encoder over graphs, model-parallel decoder):
- Host: re-encode the (static-topology) edge list as dense per-graph
  adjacency AT[g] = A[g]^T + I (A[d,s] = multiplicity of edge s->d) so GIN
  aggregation z = (I+A) @ h becomes a block-diagonal matmul. BatchNorm (eval
  mode) is folded into the following Linear. The decoder's upper-triangle
  scatter + symmetrize + diag(-10) is folded into a padded final weight
  matrix W3p[:, i*160+j] = W3[:, ut_idx(min(i,j),max(i,j))] (0 on diag,
  bias -10 on diag), sharded by output rows across cores.
- Device: 32 graphs/core through 4 GIN layers (fp32r matmuls), sum-pool via
  ones-matmul, fc -> latent; decoder d1/d2 per-core, AllGather of z2, then
  each core computes adjacency rows [20c, 20c+20) for ALL 256 graphs.
"""
import sys

for _p in ("/opt/trn_rl_repo", "/root/.axon_site/_ro/trn_rl_repo"):
    if _p not in sys.path:
        sys.path.append(_p)

import numpy as np

import concourse.bacc as bacc
import concourse.bass as bass
import concourse.tile as tile
from concourse import mybir

F32 = mybir.dt.float32
F32R = mybir.dt.float32r
AF = mybir.ActivationFunctionType

# ---- problem geometry (hardcoded; matches reference.py) ----
B = 256                 # graphs
NPG = 160               # nodes per graph
DEG = 16
N_NODES = B * NPG
N_EDGES = N_NODES * DEG
F0 = 128                # input feature dim
FH = 256                # encoder hidden
LAT = 64
HD = 512
NL_ENC = 4
N_UT = NPG * (NPG - 1) // 2
BN_EPS = 1e-5
ALPHA = 0.2             # LeakyReLU slope

N_CORES = 8
GPC = B // N_CORES      # 32 graphs per core
CH = 16                 # graphs per chunk
NCHUNK = GPC // CH      # 2
ROWS_PC = NPG // N_CORES            # 20 adjacency rows per core
NOUT = ROWS_PC * NPG                # 3200 padded output cols per core
S_HI, S_LO = 128, NPG - 128         # node split within a graph

_CACHE = {}


def _build_program():
    nc = bacc.Bacc(target_bir_lowering=False)

    # ---------------- DRAM I/O ----------------
    x_d = nc.dram_tensor("x", [GPC * NPG, F0], F32, kind="ExternalInput")
    at_d = nc.dram_tensor("at", [GPC, NPG, NPG], F32, kind="ExternalInput")
    w1_d, b1_d, w2_d, b2_d = [], [], [], []
    for l in range(NL_ENC):
        fin = F0 if l == 0 else FH
        w1_d.append(nc.dram_tensor(f"w1_{l}", [fin, FH], F32, kind="ExternalInput"))
        b1_d.append(nc.dram_tensor(f"b1_{l}", [FH, 1], F32, kind="ExternalInput"))
        w2_d.append(nc.dram_tensor(f"w2_{l}", [FH, FH], F32, kind="ExternalInput"))
        b2_d.append(nc.dram_tensor(f"b2_{l}", [1, FH], F32, kind="ExternalInput"))
    fcw_d = nc.dram_tensor("fcw", [FH, LAT], F32, kind="ExternalInput")
    fcb_d = nc.dram_tensor("fcb", [1, LAT], F32, kind="ExternalInput")
    d1w_d = nc.dram_tensor("d1w", [LAT, HD], F32, kind="ExternalInput")
    d1b_d = nc.dram_tensor("d1b", [HD, 1], F32, kind="ExternalInput")
    d2w_d = nc.dram_tensor("d2w", [HD, HD], F32, kind="ExternalInput")
    d2b_d = nc.dram_tensor("d2b", [HD, 1], F32, kind="ExternalInput")
    d3w_d = nc.dram_tensor("d3w", [HD, NOUT], F32, kind="ExternalInput")
    gsel_d = nc.dram_tensor("gsel", [128, CH * CH], F32, kind="ExternalInput")
    onesr_d = nc.dram_tensor("onesr", [1, 128], F32, kind="ExternalInput")
    d3b_d = nc.dram_tensor("d3b", [1, NOUT], F32, kind="ExternalInput")
    out_d = nc.dram_tensor("out", [B, NOUT], F32, kind="ExternalOutput")
    # collective bounce buffers (collectives can't touch I/O tensors)
    cc_in = nc.dram_tensor("cc_in", [HD, GPC], F32)
    cc_out = nc.dram_tensor("cc_out", [N_CORES * HD, GPC], F32, addr_space="Shared")

    def r(ap):
        return ap.bitcast(F32R)

    from contextlib import ExitStack
    ctx = ExitStack()
    with tile.TileContext(nc) as tc, ctx:
        wpool = ctx.enter_context(tc.tile_pool(name="weights", bufs=1))
        hpool = ctx.enter_context(tc.tile_pool(name="acts_h", bufs=2))
        zpool = ctx.enter_context(tc.tile_pool(name="acts_z", bufs=1))
        atpool = ctx.enter_context(tc.tile_pool(name="at", bufs=1))
        d3pool = ctx.enter_context(tc.tile_pool(name="d3w", bufs=2))
        opool = ctx.enter_context(tc.tile_pool(name="outstage", bufs=2))
        psZ = ctx.enter_context(tc.tile_pool(name="psZ", bufs=2, space="PSUM"))
        psA = ctx.enter_context(tc.tile_pool(name="psA", bufs=2, space="PSUM"))
        psBhi = ctx.enter_context(tc.tile_pool(name="psBhi", bufs=2, space="PSUM"))
        psBlo = ctx.enter_context(tc.tile_pool(name="psBlo", bufs=1, space="PSUM"))
        psS = ctx.enter_context(tc.tile_pool(name="psS", bufs=1, space="PSUM"))

        # ---------------- static weights to SBUF ----------------
        w1_sb, b1_sb, w2_sb, b2_sb = [], [], [], []
        for l in range(NL_ENC):
            fin = F0 if l == 0 else FH
            nk = fin // 128
            t = wpool.tile([128, nk, FH], F32R, tag=f"w1_{l}")
            nc.sync.dma_start(out=t[:], in_=bass.AP(
                w1_d[l], 0, [[FH, 128], [128 * FH, nk], [1, FH]]).bitcast(F32R))
            w1_sb.append(t)
            tb = wpool.tile([128, FH // 128, 1], F32, tag=f"b1_{l}")
            nc.sync.dma_start(out=tb[:], in_=bass.AP(
                b1_d[l], 0, [[1, 128], [128, FH // 128], [1, 1]]))
            b1_sb.append(tb)
            t2 = wpool.tile([128, FH // 128, FH], F32R, tag=f"w2_{l}")
            nc.sync.dma_start(out=t2[:], in_=bass.AP(
                w2_d[l], 0, [[FH, 128], [128 * FH, FH // 128], [1, FH]]).bitcast(F32R))
            w2_sb.append(t2)
            t3 = wpool.tile([1, FH], F32R, tag=f"b2_{l}")
            nc.sync.dma_start(out=t3[:], in_=r(b2_d[l][:, :]))
            b2_sb.append(t3)
        fcw_sb = wpool.tile([128, FH // 128, LAT], F32R, tag="fcw")
        nc.sync.dma_start(out=fcw_sb[:], in_=bass.AP(
            fcw_d, 0, [[LAT, 128], [128 * LAT, FH // 128], [1, LAT]]).bitcast(F32R))
        fcb_sb = wpool.tile([1, LAT], F32R, tag="fcb")
        nc.sync.dma_start(out=fcb_sb[:], in_=r(fcb_d[:, :]))
        d1w_sb = wpool.tile([LAT, HD], F32R, tag="d1w")
        nc.sync.dma_start(out=d1w_sb[:], in_=r(d1w_d[:, :]))
        d1b_sb = wpool.tile([128, HD // 128, 1], F32, tag="d1b")
        nc.sync.dma_start(out=d1b_sb[:], in_=bass.AP(
            d1b_d, 0, [[1, 128], [128, HD // 128], [1, 1]]))
        d2w_sb = wpool.tile([128, HD // 128, HD], F32R, tag="d2w")
        nc.sync.dma_start(out=d2w_sb[:], in_=bass.AP(
            d2w_d, 0, [[HD, 128], [128 * HD, HD // 128], [1, HD]]).bitcast(F32R))
        d2b_sb = wpool.tile([128, HD // 128, 1], F32, tag="d2b")
        nc.sync.dma_start(out=d2b_sb[:], in_=bass.AP(
            d2b_d, 0, [[1, 128], [128, HD // 128], [1, 1]]))
        d3b_sb = wpool.tile([1, NOUT], F32R, tag="d3b")
        nc.sync.dma_start(out=d3b_sb[:], in_=r(d3b_d[:, :]))

        alpha_ap = wpool.tile([128, 1], F32, tag="alpha")
        nc.vector.memset(alpha_ap[:], ALPHA)
        gsel_sb = wpool.tile([128, CH * CH], F32R, tag="gsel")
        nc.sync.dma_start(out=gsel_sb[:], in_=r(gsel_d[:, :]))
        ones_row = wpool.tile([1, 128], F32R, tag="ones_row")
        nc.sync.dma_start(out=ones_row[:], in_=r(onesr_d[:, :]))

        pooledT = zpool.tile([128, FH // 128, GPC], F32R, tag="pooledT")

        # ---------------- encoder, per chunk of 16 graphs ----------------
        for ch in range(NCHUNK):
            g0 = ch * CH
            # load x chunk into layer-0 h buffers (node-major, graph-blocked)
            h_hi = hpool.tile([128, CH, FH], F32R, tag="h_hi")
            h_lo = hpool.tile([S_LO, CH, FH], F32R, tag="h_lo")
            nc.sync.dma_start(out=h_hi[:, :, 0:F0], in_=bass.AP(
                x_d, g0 * NPG * F0,
                [[F0, 128], [NPG * F0, CH], [1, F0]]).bitcast(F32R))
            nc.sync.dma_start(out=h_lo[:, :, 0:F0], in_=bass.AP(
                x_d, (g0 * NPG + S_HI) * F0,
                [[F0, S_LO], [NPG * F0, CH], [1, F0]]).bitcast(F32R))
            at_hi = atpool.tile([128, CH, NPG], F32R, tag="at_hi")
            at_lo = atpool.tile([S_LO, CH, NPG], F32R, tag="at_lo")
            nc.sync.dma_start(out=at_hi[:], in_=bass.AP(
                at_d, g0 * NPG * NPG,
                [[NPG, 128], [NPG * NPG, CH], [1, NPG]]).bitcast(F32R))
            nc.sync.dma_start(out=at_lo[:], in_=bass.AP(
                at_d, g0 * NPG * NPG + S_HI * NPG,
                [[NPG, S_LO], [NPG * NPG, CH], [1, NPG]]).bitcast(F32R))

            for l in range(NL_ENC):
                fin = F0 if l == 0 else FH
                nm = fin // 128   # M tiles for zT / K tiles for MLP1
                # ---- zT = h^T AT^ (aggregation incl. +I), feature-major ----
                zT = zpool.tile([128, FH // 128, CH * NPG], F32R, tag="zT")
                for g in range(CH):
                    for m in range(nm):
                        pz = psZ.tile([128, NPG], F32, tag="psZ")
                        nc.tensor.matmul(
                            pz[:], lhsT=h_hi[:, g, m * 128:(m + 1) * 128],
                            rhs=at_hi[:, g, :], start=True, stop=False)
                        nc.tensor.matmul(
                            pz[:], lhsT=h_lo[:, g, m * 128:(m + 1) * 128],
                            rhs=at_lo[:, g, :], start=False, stop=True)
                        nc.vector.tensor_copy(
                            zT[:, m, g * NPG:(g + 1) * NPG], pz[:])
                # ---- y1T = Prelu(W1^T z + b1), feature-major ----
                y1T = zpool.tile([128, FH // 128, CH * NPG], F32R, tag="y1T")
                NW = 512
                nn = (CH * NPG) // NW  # 5
                for m in range(FH // 128):
                    for n in range(nn):
                        pa = psA.tile([128, NW], F32, tag="psA")
                        for k in range(nm):
                            nc.tensor.matmul(
                                pa[:], lhsT=w1_sb[l][:, k, m * 128:(m + 1) * 128],
                                rhs=zT[:, k, n * NW:(n + 1) * NW],
                                start=(k == 0), stop=(k == nm - 1))
                        nc.scalar.activation(
                            y1T[:, m, n * NW:(n + 1) * NW], pa[:],
                            AF.Prelu, bias=b1_sb[l][:, m, 0:1], scale=1.0,
                            alpha=alpha_ap[:])
                # ---- h_next = Prelu(y1 W2 + b2), node-major (per graph) ----
                hn_hi = hpool.tile([128, CH, FH], F32R, tag="h_hi")
                hn_lo = hpool.tile([S_LO, CH, FH], F32R, tag="h_lo")
                for g in range(CH):
                    c0 = g * NPG
                    pb = psBhi.tile([128, FH], F32, tag="psB_hi")
                    for k in range(FH // 128):
                        nc.tensor.matmul(
                            pb[:], lhsT=y1T[:, k, c0:c0 + 128],
                            rhs=w2_sb[l][:, k, :], start=(k == 0), stop=False)
                    nc.tensor.matmul(
                        pb[:], lhsT=ones_row[:, 0:128], rhs=b2_sb[l][:],
                        start=False, stop=True)
                    nc.scalar.activation(hn_hi[:, g, :], pb[:],
                                         AF.Prelu, alpha=alpha_ap[:])
                    pc = psBlo.tile([S_LO, FH], F32, tag="psB_lo")
                    for k in range(FH // 128):
                        nc.tensor.matmul(
                            pc[:], lhsT=y1T[:, k, c0 + S_HI:c0 + NPG],
                            rhs=w2_sb[l][:, k, :], start=(k == 0), stop=False)
                    nc.tensor.matmul(
                        pc[:], lhsT=ones_row[:, 0:S_LO], rhs=b2_sb[l][:],
                        start=False, stop=True)
                    nc.scalar.activation(hn_lo[:, g, :], pc[:],
                                         AF.Prelu, alpha=alpha_ap[0:S_LO, :])
                h_hi, h_lo = hn_hi, hn_lo

            # ---- sum-pool h4 over nodes (per graph) -> pooledT cols ----
            for m in range(FH // 128):
                pp = psS.tile([128, CH], F32, tag="psSmall")
                for g in range(CH):
                    nc.tensor.matmul(
                        pp[:], lhsT=h_hi[:, g, m * 128:(m + 1) * 128],
                        rhs=gsel_sb[:, g * CH:(g + 1) * CH],
                        start=(g == 0), stop=False)
                    nc.tensor.matmul(
                        pp[:], lhsT=h_lo[:, g, m * 128:(m + 1) * 128],
                        rhs=gsel_sb[0:S_LO, g * CH:(g + 1) * CH],
                        start=False, stop=(g == CH - 1))
                nc.vector.tensor_copy(pooledT[:, m, g0:g0 + CH], pp[:])

        # ---------------- latent + decoder d1/d2 (per-core graphs) ----------
        latT = zpool.tile([LAT, GPC], F32R, tag="latT")
        pl = psS.tile([LAT, GPC], F32, tag="psSmall")
        for k in range(FH // 128):
            nc.tensor.matmul(pl[:], lhsT=fcw_sb[:, k, :], rhs=pooledT[:, k, :],
                             start=(k == 0), stop=False)
        nc.tensor.matmul(pl[:], lhsT=fcb_sb[:], rhs=ones_row[:, 0:GPC],
                         start=False, stop=True)
        nc.vector.tensor_copy(latT[:], pl[:])

        ydT = zpool.tile([128, HD // 128, GPC], F32R, tag="ydT")
        for m in range(HD // 128):
            pd = psS.tile([128, GPC], F32, tag="psSmall")
            nc.tensor.matmul(pd[:], lhsT=d1w_sb[:, m * 128:(m + 1) * 128],
                             rhs=latT[:], start=True, stop=True)
            nc.scalar.activation(ydT[:, m, :], pd[:], AF.Relu,
                                 bias=d1b_sb[:, m, 0:1])
        z2T = zpool.tile([128, HD // 128, GPC], F32, tag="z2T")
        for m in range(HD // 128):
            pd = psS.tile([128, GPC], F32, tag="psSmall")
            for k in range(HD // 128):
                nc.tensor.matmul(pd[:], lhsT=d2w_sb[:, k, m * 128:(m + 1) * 128],
                                 rhs=ydT[:, k, :],
                                 start=(k == 0), stop=(k == HD // 128 - 1))
            nc.scalar.activation(z2T[:, m, :], pd[:], AF.Relu,
                                 bias=d2b_sb[:, m, 0:1])

        # ---------------- all-gather z2 across cores ----------------
        nc.sync.dma_start(out=bass.AP(
            cc_in, 0, [[GPC, 128], [128 * GPC, HD // 128], [1, GPC]]),
            in_=z2T[:])
        nc.gpsimd.collective_compute(
            "AllGather", mybir.AluOpType.bypass,
            replica_groups=[list(range(N_CORES))],
            ins=[cc_in.ap().opt()], outs=[cc_out.ap().opt()])
        z2all = zpool.tile([128, HD // 128, B], F32R, tag="z2all")
        for c in range(N_CORES):
            for k in range(HD // 128):
                nc.sync.dma_start(
                    out=z2all[:, k, c * GPC:(c + 1) * GPC],
                    in_=bass.AP(cc_out, (c * HD + k * 128) * GPC,
                                [[GPC, 128], [1, GPC]]).bitcast(F32R))

        # ---------------- model-parallel d3: adjacency row-slab ----------
        NW3 = 512
        n_n3 = (NOUT + NW3 - 1) // NW3
        for n in range(n_n3):
            c0 = n * NW3
            cw = min(NW3, NOUT - c0)
            w3t = d3pool.tile([128, HD // 128, NW3], F32R, tag="w3")
            nc.sync.dma_start(out=w3t[:, :, 0:cw], in_=bass.AP(
                d3w_d, c0, [[NOUT, 128], [128 * NOUT, HD // 128], [1, cw]]
            ).bitcast(F32R))
            for m in range(B // 128):
                po = psA.tile([128, NW3], F32, tag="psA")
                for k in range(HD // 128):
                    nc.tensor.matmul(
                        po[:, 0:cw], lhsT=z2all[:, k, m * 128:(m + 1) * 128],
                        rhs=w3t[:, k, 0:cw],
                        start=(k == 0), stop=False)
                nc.tensor.matmul(
                    po[:, 0:cw], lhsT=ones_row[:], rhs=d3b_sb[:, c0:c0 + cw],
                    start=False, stop=True)
                ot = opool.tile([128, NW3], F32, tag="ostage")
                nc.vector.tensor_copy(ot[:, 0:cw], po[:, 0:cw])
                nc.sync.dma_start(
                    out=out_d[m * 128:(m + 1) * 128, c0:c0 + cw],
                    in_=ot[:, 0:cw])

    nc.compile()
    return nc


def _prep_host(x, edge_index, batch, params):
    """Shard inputs + fold BN / build dense AT / pad decoder weights."""
    x = np.asarray(x, dtype=np.float32)
    src = np.asarray(edge_index[0], dtype=np.int64)
    dst = np.asarray(edge_index[1], dtype=np.int64)

    # dense AT[g, s, d] = #edges s->d in graph g, + I (GIN eps=0 self term)
    g_of_e = src // NPG
    code = (g_of_e * NPG + (src % NPG)) * NPG + (dst % NPG)
    at = np.bincount(code, minlength=B * NPG * NPG).reshape(B, NPG, NPG)
    at = at.astype(np.float32)
    at += np.eye(NPG, dtype=np.float32)[None]

    inv_std = np.float32(1.0 / np.sqrt(np.float32(1.0 + BN_EPS)))

    def A(v):
        return np.asarray(v, dtype=np.float32)

    wm = {}
    for l, lyr in enumerate(params["enc"]):
        W1, b1 = A(lyr["W1"]), A(lyr["b1"])
        g, bt = A(lyr["g"]) * inv_std, A(lyr["bt"])
        W2, b2 = A(lyr["W2"]), A(lyr["b2"])
        wm[f"w1_{l}"] = np.ascontiguousarray(W1)
        wm[f"b1_{l}"] = np.ascontiguousarray(b1.reshape(FH, 1))
        wm[f"w2_{l}"] = np.ascontiguousarray(W2 * g[:, None])
        wm[f"b2_{l}"] = np.ascontiguousarray((b2 + bt @ W2).reshape(1, FH))
    bn_g = A(params["bn_g"]) * inv_std
    bn_b = A(params["bn_b"])
    fcW, fcb = A(params["fc_W"]), A(params["fc_b"])
    wm["fcw"] = np.ascontiguousarray(fcW * bn_g[:, None])
    wm["fcb"] = np.ascontiguousarray((fcb + bn_b @ fcW).reshape(1, LAT))
    dec = params["dec"]
    wm["d1w"] = np.ascontiguousarray(A(dec[0]["W"]))
    wm["d1b"] = np.ascontiguousarray(A(dec[0]["b"]).reshape(HD, 1))
    wm["d2w"] = np.ascontiguousarray(A(dec[1]["W"]))
    wm["d2b"] = np.ascontiguousarray(A(dec[1]["b"]).reshape(HD, 1))
    wm["gsel"] = np.ascontiguousarray(
        np.tile(np.eye(CH, dtype=np.float32).reshape(1, CH * CH), (128, 1)))
    wm["onesr"] = np.ones((1, 128), dtype=np.float32)

    # padded final layer: column (i,j) = W3[:, ut_idx] (sym), 0 on diag
    W3, b3 = A(dec[2]["W"]), A(dec[2]["b"])
    iu0, iu1 = np.triu_indices(NPG, k=1)
    ut_of = np.zeros((NPG, NPG), dtype=np.int64)
    ut_of[iu0, iu1] = np.arange(N_UT)
    ut_of[iu1, iu0] = np.arange(N_UT)
    Wp = np.zeros((HD, NPG * NPG), dtype=np.float32)
    bp = np.full((NPG * NPG,), -10.0, dtype=np.float32)
    off = np.where(np.eye(NPG, dtype=bool).ravel(), -1, ut_of.ravel())
    nz = off >= 0
    Wp[:, nz] = W3[:, off[nz]]
    bp[nz] = b3[off[nz]]
    Wp = Wp.reshape(HD, NPG, NPG)
    bp = bp.reshape(NPG, NPG)

    in_maps = []
    for c in range(N_CORES):
        m = dict(wm)
        gs = c * GPC
        m["x"] = np.ascontiguousarray(x[gs * NPG:(gs + GPC) * NPG])
        m["at"] = np.ascontiguousarray(at[gs:gs + GPC])  # [g, s, d]
        r0 = c * ROWS_PC
        m["d3w"] = np.ascontiguousarray(
            Wp[:, r0:r0 + ROWS_PC, :].reshape(HD, NOUT))
        m["d3b"] = np.ascontiguousarray(
            bp[r0:r0 + ROWS_PC, :].reshape(1, NOUT))
        in_maps.append(m)
    return in_maps


def kernel(x, edge_index, batch, params):
    if "nc" not in _CACHE:
        _CACHE["nc"] = _build_program()
    nc = _CACHE["nc"]
    in_maps = _prep_host(x, edge_index, batch, params)

    from concourse.bass_utils import run_bass_kernel_spmd
    res = run_bass_kernel_spmd(nc, in_maps, list(range(N_CORES)))
    # core c holds adjacency rows [c*20, c*20+20) for all graphs
    slabs = [res.results[c]["out"].reshape(B, ROWS_PC, NPG)
             for c in range(N_CORES)]
    return np.ascontiguousarray(np.concatenate(slabs, axis=1))


# revision 8
# speedup vs baseline: 1.1781x; 1.1781x over previous
"""GIN autoencoder forward pass on 8 Trainium2 NeuronCores.

Strategy (data-parallel encoder over graphs, model-parallel decoder):
- Host: re-encode the (static-topology) edge list as dense per-graph
  adjacency AT[g] = A[g]^T + I (A[d,s] = multiplicity of edge s->d) so GIN
  aggregation z = (I+A) @ h becomes a block-diagonal matmul. BatchNorm (eval
  mode) is folded into the following Linear. The decoder's upper-triangle
  scatter + symmetrize + diag(-10) is folded into a padded final weight
  matrix W3p[:, i*160+j] = W3[:, ut_idx(min(i,j),max(i,j))] (0 on diag,
  bias -10 on diag), sharded by output rows across cores.
- Device: 32 graphs/core through 4 GIN layers (fp32r matmuls), sum-pool via
  ones-matmul, fc -> latent; decoder d1/d2 per-core, AllGather of z2, then
  each core computes adjacency rows [20c, 20c+20) for ALL 256 graphs.
"""
import sys

for _p in ("/opt/trn_rl_repo", "/root/.axon_site/_ro/trn_rl_repo"):
    if _p not in sys.path:
        sys.path.append(_p)

import numpy as np

import concourse.bacc as bacc
import concourse.bass as bass
import concourse.tile as tile
from concourse import mybir

F32 = mybir.dt.float32
F32R = mybir.dt.float32r
AF = mybir.ActivationFunctionType

# ---- problem geometry (hardcoded; matches reference.py) ----
B = 256                 # graphs
NPG = 160               # nodes per graph
DEG = 16
N_NODES = B * NPG
N_EDGES = N_NODES * DEG
F0 = 128                # input feature dim
FH = 256                # encoder hidden
LAT = 64
HD = 512
NL_ENC = 4
N_UT = NPG * (NPG - 1) // 2
BN_EPS = 1e-5
ALPHA = 0.2             # LeakyReLU slope

N_CORES = 8
GPC = B // N_CORES      # 32 graphs per core
CH = 16                 # graphs per chunk
NCHUNK = GPC // CH      # 2
ROWS_PC = NPG // N_CORES            # 20 adjacency rows per core
NOUT = ROWS_PC * NPG                # 3200 padded output cols per core
S_HI, S_LO = 128, NPG - 128         # node split within a graph
ATP = 256                           # AT padded cols (fp32r fast path)

_CACHE = {}


def _build_program():
    nc = bacc.Bacc(target_bir_lowering=False)

    # ---------------- DRAM I/O ----------------
    x_d = nc.dram_tensor("x", [GPC * NPG, F0], F32, kind="ExternalInput")
    at_d = nc.dram_tensor("at", [GPC, NPG, ATP], F32, kind="ExternalInput")
    w1_d, b1_d, w2_d, b2_d = [], [], [], []
    for l in range(NL_ENC):
        fin = F0 if l == 0 else FH
        w1_d.append(nc.dram_tensor(f"w1_{l}", [fin, FH], F32, kind="ExternalInput"))
        b1_d.append(nc.dram_tensor(f"b1_{l}", [FH, 1], F32, kind="ExternalInput"))
        w2_d.append(nc.dram_tensor(f"w2_{l}", [FH, FH], F32, kind="ExternalInput"))
        b2_d.append(nc.dram_tensor(f"b2_{l}", [1, FH], F32, kind="ExternalInput"))
    fcw_d = nc.dram_tensor("fcw", [FH, LAT], F32, kind="ExternalInput")
    fcb_d = nc.dram_tensor("fcb", [1, LAT], F32, kind="ExternalInput")
    d1w_d = nc.dram_tensor("d1w", [LAT, HD], F32, kind="ExternalInput")
    d1b_d = nc.dram_tensor("d1b", [HD, 1], F32, kind="ExternalInput")
    d2w_d = nc.dram_tensor("d2w", [HD, HD], F32, kind="ExternalInput")
    d2b_d = nc.dram_tensor("d2b", [HD, 1], F32, kind="ExternalInput")
    d3w_d = nc.dram_tensor("d3w", [HD, NOUT], F32, kind="ExternalInput")
    gsel_d = nc.dram_tensor("gsel", [128, CH * CH], F32, kind="ExternalInput")
    onesr_d = nc.dram_tensor("onesr", [1, 128], F32, kind="ExternalInput")
    d3b_d = nc.dram_tensor("d3b", [1, NOUT], F32, kind="ExternalInput")
    out_d = nc.dram_tensor("out", [B, NOUT], F32, kind="ExternalOutput")
    # collective bounce buffers (collectives can't touch I/O tensors)
    cc_in = nc.dram_tensor("cc_in", [HD, GPC], F32)
    cc_out = nc.dram_tensor("cc_out", [N_CORES * HD, GPC], F32, addr_space="Shared")

    def r(ap):
        return ap.bitcast(F32R)

    from contextlib import ExitStack
    ctx = ExitStack()
    with tile.TileContext(nc) as tc, ctx:
        wpool = ctx.enter_context(tc.tile_pool(name="weights", bufs=1))
        hpool = ctx.enter_context(tc.tile_pool(name="acts_h", bufs=2))
        zpool = ctx.enter_context(tc.tile_pool(name="acts_z", bufs=1))
        atpool = ctx.enter_context(tc.tile_pool(name="at", bufs=1))
        d3pool = ctx.enter_context(tc.tile_pool(name="d3w", bufs=2))
        opool = ctx.enter_context(tc.tile_pool(name="outstage", bufs=2))
        psZ = ctx.enter_context(tc.tile_pool(name="psZ", bufs=2, space="PSUM"))
        psA = ctx.enter_context(tc.tile_pool(name="psA", bufs=2, space="PSUM"))
        psBhi = ctx.enter_context(tc.tile_pool(name="psBhi", bufs=2, space="PSUM"))
        psBlo = ctx.enter_context(tc.tile_pool(name="psBlo", bufs=1, space="PSUM"))
        psS = ctx.enter_context(tc.tile_pool(name="psS", bufs=1, space="PSUM"))

        # ---------------- static weights to SBUF ----------------
        w1_sb, b1_sb, w2_sb, b2_sb = [], [], [], []
        for l in range(NL_ENC):
            fin = F0 if l == 0 else FH
            nk = fin // 128
            t = wpool.tile([128, nk, FH], F32R, tag=f"w1_{l}")
            nc.sync.dma_start(out=t[:], in_=bass.AP(
                w1_d[l], 0, [[FH, 128], [128 * FH, nk], [1, FH]]).bitcast(F32R))
            w1_sb.append(t)
            tb = wpool.tile([128, FH // 128, 1], F32, tag=f"b1_{l}")
            nc.sync.dma_start(out=tb[:], in_=bass.AP(
                b1_d[l], 0, [[1, 128], [128, FH // 128], [1, 1]]))
            b1_sb.append(tb)
            t2 = wpool.tile([128, FH // 128, FH], F32R, tag=f"w2_{l}")
            nc.sync.dma_start(out=t2[:], in_=bass.AP(
                w2_d[l], 0, [[FH, 128], [128 * FH, FH // 128], [1, FH]]).bitcast(F32R))
            w2_sb.append(t2)
            t3 = wpool.tile([128, FH], F32, tag=f"b2_{l}")
            nc.sync.dma_start(out=t3[:], in_=bass.AP(
                b2_d[l], 0, [[0, 128], [1, FH]]))
            b2_sb.append(t3)
        fcw_sb = wpool.tile([128, FH // 128, LAT], F32R, tag="fcw")
        nc.sync.dma_start(out=fcw_sb[:], in_=bass.AP(
            fcw_d, 0, [[LAT, 128], [128 * LAT, FH // 128], [1, LAT]]).bitcast(F32R))
        fcb_sb = wpool.tile([1, LAT], F32R, tag="fcb")
        nc.sync.dma_start(out=fcb_sb[:], in_=r(fcb_d[:, :]))
        d1w_sb = wpool.tile([LAT, HD], F32R, tag="d1w")
        nc.sync.dma_start(out=d1w_sb[:], in_=r(d1w_d[:, :]))
        d1b_sb = wpool.tile([128, HD // 128, 1], F32, tag="d1b")
        nc.sync.dma_start(out=d1b_sb[:], in_=bass.AP(
            d1b_d, 0, [[1, 128], [128, HD // 128], [1, 1]]))
        d2w_sb = wpool.tile([128, HD // 128, HD], F32R, tag="d2w")
        nc.sync.dma_start(out=d2w_sb[:], in_=bass.AP(
            d2w_d, 0, [[HD, 128], [128 * HD, HD // 128], [1, HD]]).bitcast(F32R))
        d2b_sb = wpool.tile([128, HD // 128, 1], F32, tag="d2b")
        nc.sync.dma_start(out=d2b_sb[:], in_=bass.AP(
            d2b_d, 0, [[1, 128], [128, HD // 128], [1, 1]]))
        d3b_sb = wpool.tile([128, NOUT], F32, tag="d3b")
        nc.sync.dma_start(out=d3b_sb[:], in_=bass.AP(
            d3b_d, 0, [[0, 128], [1, NOUT]]))

        alpha_ap = wpool.tile([128, 1], F32, tag="alpha")
        nc.vector.memset(alpha_ap[:], ALPHA)
        gsel_sb = wpool.tile([128, CH * CH], F32R, tag="gsel")
        nc.sync.dma_start(out=gsel_sb[:], in_=r(gsel_d[:, :]))
        ones_row = wpool.tile([1, 128], F32R, tag="ones_row")
        nc.sync.dma_start(out=ones_row[:], in_=r(onesr_d[:, :]))

        pooledT = zpool.tile([128, FH // 128, GPC], F32R, tag="pooledT")

        # ---------------- encoder, per chunk of 16 graphs ----------------
        for ch in range(NCHUNK):
            g0 = ch * CH
            # load x chunk into layer-0 h buffers (node-major, graph-blocked)
            h_hi = hpool.tile([128, CH, FH], F32R, tag="h_hi")
            h_lo = hpool.tile([S_LO, CH, FH], F32R, tag="h_lo")
            nc.sync.dma_start(out=h_hi[:, :, 0:F0], in_=bass.AP(
                x_d, g0 * NPG * F0,
                [[F0, 128], [NPG * F0, CH], [1, F0]]).bitcast(F32R))
            nc.sync.dma_start(out=h_lo[:, :, 0:F0], in_=bass.AP(
                x_d, (g0 * NPG + S_HI) * F0,
                [[F0, S_LO], [NPG * F0, CH], [1, F0]]).bitcast(F32R))
            at_hi = atpool.tile([128, CH, ATP], F32R, tag="at_hi")
            at_lo = atpool.tile([S_LO, CH, ATP], F32R, tag="at_lo")
            nc.sync.dma_start(out=at_hi[:], in_=bass.AP(
                at_d, g0 * NPG * ATP,
                [[ATP, 128], [NPG * ATP, CH], [1, ATP]]).bitcast(F32R))
            nc.sync.dma_start(out=at_lo[:], in_=bass.AP(
                at_d, g0 * NPG * ATP + S_HI * ATP,
                [[ATP, S_LO], [NPG * ATP, CH], [1, ATP]]).bitcast(F32R))

            for l in range(NL_ENC):
                fin = F0 if l == 0 else FH
                nm = fin // 128   # M tiles for zT / K tiles for MLP1
                # ---- zT = h^T AT^ (aggregation incl. +I), feature-major ----
                zT = zpool.tile([128, FH // 128, CH * NPG], F32R, tag="zT")
                for g in range(CH):
                    for m in range(nm):
                        pz = psZ.tile([128, ATP], F32, tag="psZ")
                        nc.tensor.matmul(
                            pz[:], lhsT=h_hi[:, g, m * 128:(m + 1) * 128],
                            rhs=at_hi[:, g, :], start=True, stop=False)
                        nc.tensor.matmul(
                            pz[:], lhsT=h_lo[:, g, m * 128:(m + 1) * 128],
                            rhs=at_lo[:, g, :], start=False, stop=True)
                        nc.vector.tensor_copy(
                            zT[:, m, g * NPG:(g + 1) * NPG], pz[:, 0:NPG])
                # ---- y1T = Prelu(W1^T z + b1), feature-major ----
                y1T = zpool.tile([128, FH // 128, CH * NPG], F32R, tag="y1T")
                NW = 512
                nn = (CH * NPG) // NW  # 5
                for m in range(FH // 128):
                    for n in range(nn):
                        pa = psA.tile([128, NW], F32, tag="psA")
                        for k in range(nm):
                            nc.tensor.matmul(
                                pa[:], lhsT=w1_sb[l][:, k, m * 128:(m + 1) * 128],
                                rhs=zT[:, k, n * NW:(n + 1) * NW],
                                start=(k == 0), stop=(k == nm - 1))
                        nc.scalar.activation(
                            y1T[:, m, n * NW:(n + 1) * NW], pa[:],
                            AF.Prelu, bias=b1_sb[l][:, m, 0:1], scale=1.0,
                            alpha=alpha_ap[:])
                # ---- h_next = Prelu(y1 W2 + b2), node-major (per graph) ----
                hn_hi = hpool.tile([128, CH, FH], F32R, tag="h_hi")
                hn_lo = hpool.tile([S_LO, CH, FH], F32R, tag="h_lo")
                for g in range(CH):
                    c0 = g * NPG
                    pb = psBhi.tile([128, FH], F32, tag="psB_hi")
                    for k in range(FH // 128):
                        nc.tensor.matmul(
                            pb[:], lhsT=y1T[:, k, c0:c0 + 128],
                            rhs=w2_sb[l][:, k, :], start=(k == 0),
                            stop=(k == FH // 128 - 1))
                    nc.vector.tensor_tensor(
                        out=hn_hi[:, g, :], in0=pb[:],
                        in1=b2_sb[l][:],
                        op=mybir.AluOpType.add)
                    nc.scalar.activation(hn_hi[:, g, :], hn_hi[:, g, :],
                                         AF.Prelu, alpha=alpha_ap[:])
                    pc = psBlo.tile([S_LO, FH], F32, tag="psB_lo")
                    for k in range(FH // 128):
                        nc.tensor.matmul(
                            pc[:], lhsT=y1T[:, k, c0 + S_HI:c0 + NPG],
                            rhs=w2_sb[l][:, k, :], start=(k == 0),
                            stop=(k == FH // 128 - 1))
                    nc.vector.tensor_tensor(
                        out=hn_lo[:, g, :], in0=pc[:],
                        in1=b2_sb[l][0:S_LO, :],
                        op=mybir.AluOpType.add)
                    nc.scalar.activation(hn_lo[:, g, :], hn_lo[:, g, :],
                                         AF.Prelu, alpha=alpha_ap[0:S_LO, :])
                h_hi, h_lo = hn_hi, hn_lo

            # ---- sum-pool h4 over nodes (per graph) -> pooledT cols ----
            for m in range(FH // 128):
                pp = psS.tile([128, CH], F32, tag="psSmall")
                for g in range(CH):
                    nc.tensor.matmul(
                        pp[:], lhsT=h_hi[:, g, m * 128:(m + 1) * 128],
                        rhs=gsel_sb[:, g * CH:(g + 1) * CH],
                        start=(g == 0), stop=False)
                    nc.tensor.matmul(
                        pp[:], lhsT=h_lo[:, g, m * 128:(m + 1) * 128],
                        rhs=gsel_sb[0:S_LO, g * CH:(g + 1) * CH],
                        start=False, stop=(g == CH - 1))
                nc.vector.tensor_copy(pooledT[:, m, g0:g0 + CH], pp[:])

        # ---------------- latent + decoder d1/d2 (per-core graphs) ----------
        latT = zpool.tile([LAT, GPC], F32R, tag="latT")
        pl = psS.tile([LAT, GPC], F32, tag="psSmall")
        for k in range(FH // 128):
            nc.tensor.matmul(pl[:], lhsT=fcw_sb[:, k, :], rhs=pooledT[:, k, :],
                             start=(k == 0), stop=False)
        nc.tensor.matmul(pl[:], lhsT=fcb_sb[:], rhs=ones_row[:, 0:GPC],
                         start=False, stop=True)
        nc.vector.tensor_copy(latT[:], pl[:])

        ydT = zpool.tile([128, HD // 128, GPC], F32R, tag="ydT")
        for m in range(HD // 128):
            pd = psS.tile([128, GPC], F32, tag="psSmall")
            nc.tensor.matmul(pd[:], lhsT=d1w_sb[:, m * 128:(m + 1) * 128],
                             rhs=latT[:], start=True, stop=True)
            nc.scalar.activation(ydT[:, m, :], pd[:], AF.Relu,
                                 bias=d1b_sb[:, m, 0:1])
        z2T = zpool.tile([128, HD // 128, GPC], F32, tag="z2T")
        for m in range(HD // 128):
            pd = psS.tile([128, GPC], F32, tag="psSmall")
            for k in range(HD // 128):
                nc.tensor.matmul(pd[:], lhsT=d2w_sb[:, k, m * 128:(m + 1) * 128],
                                 rhs=ydT[:, k, :],
                                 start=(k == 0), stop=(k == HD // 128 - 1))
            nc.scalar.activation(z2T[:, m, :], pd[:], AF.Relu,
                                 bias=d2b_sb[:, m, 0:1])

        # ---------------- all-gather z2 across cores ----------------
        nc.sync.dma_start(out=bass.AP(
            cc_in, 0, [[GPC, 128], [128 * GPC, HD // 128], [1, GPC]]),
            in_=z2T[:])
        nc.gpsimd.collective_compute(
            "AllGather", mybir.AluOpType.bypass,
            replica_groups=[list(range(N_CORES))],
            ins=[cc_in.ap().opt()], outs=[cc_out.ap().opt()])
        z2all = zpool.tile([128, HD // 128, B], F32R, tag="z2all")
        for c in range(N_CORES):
            for k in range(HD // 128):
                nc.sync.dma_start(
                    out=z2all[:, k, c * GPC:(c + 1) * GPC],
                    in_=bass.AP(cc_out, (c * HD + k * 128) * GPC,
                                [[GPC, 128], [1, GPC]]).bitcast(F32R))

        # ---------------- model-parallel d3: adjacency row-slab ----------
        NW3 = 512
        n_n3 = (NOUT + NW3 - 1) // NW3
        for n in range(n_n3):
            c0 = n * NW3
            cw = min(NW3, NOUT - c0)
            w3t = d3pool.tile([128, HD // 128, NW3], F32R, tag="w3")
            nc.sync.dma_start(out=w3t[:, :, 0:cw], in_=bass.AP(
                d3w_d, c0, [[NOUT, 128], [128 * NOUT, HD // 128], [1, cw]]
            ).bitcast(F32R))
            for m in range(B // 128):
                po = psA.tile([128, NW3], F32, tag="psA")
                for k in range(HD // 128):
                    nc.tensor.matmul(
                        po[:, 0:cw], lhsT=z2all[:, k, m * 128:(m + 1) * 128],
                        rhs=w3t[:, k, 0:cw],
                        start=(k == 0), stop=(k == HD // 128 - 1))
                ot = opool.tile([128, NW3], F32, tag="ostage")
                nc.vector.tensor_tensor(
                    out=ot[:, 0:cw], in0=po[:, 0:cw],
                    in1=d3b_sb[:, c0:c0 + cw],
                    op=mybir.AluOpType.add)
                nc.sync.dma_start(
                    out=out_d[m * 128:(m + 1) * 128, c0:c0 + cw],
                    in_=ot[:, 0:cw])

    nc.compile()
    return nc


def _prep_host(x, edge_index, batch, params):
    """Shard inputs + fold BN / build dense AT / pad decoder weights."""
    x = np.asarray(x, dtype=np.float32)
    src = np.asarray(edge_index[0], dtype=np.int64)
    dst = np.asarray(edge_index[1], dtype=np.int64)

    # dense AT[g, s, d] = #edges s->d in graph g, + I (GIN eps=0 self term)
    g_of_e = src // NPG
    code = (g_of_e * NPG + (src % NPG)) * NPG + (dst % NPG)
    at = np.bincount(code, minlength=B * NPG * NPG).reshape(B, NPG, NPG)
    at = at.astype(np.float32)
    at += np.eye(NPG, dtype=np.float32)[None]
    at = np.concatenate(
        [at, np.zeros((B, NPG, ATP - NPG), np.float32)], axis=2)

    inv_std = np.float32(1.0 / np.sqrt(np.float32(1.0 + BN_EPS)))

    def A(v):
        return np.asarray(v, dtype=np.float32)

    wm = {}
    for l, lyr in enumerate(params["enc"]):
        W1, b1 = A(lyr["W1"]), A(lyr["b1"])
        g, bt = A(lyr["g"]) * inv_std, A(lyr["bt"])
        W2, b2 = A(lyr["W2"]), A(lyr["b2"])
        wm[f"w1_{l}"] = np.ascontiguousarray(W1)
        wm[f"b1_{l}"] = np.ascontiguousarray(b1.reshape(FH, 1))
        wm[f"w2_{l}"] = np.ascontiguousarray(W2 * g[:, None])
        wm[f"b2_{l}"] = np.ascontiguousarray((b2 + bt @ W2).reshape(1, FH))
    bn_g = A(params["bn_g"]) * inv_std
    bn_b = A(params["bn_b"])
    fcW, fcb = A(params["fc_W"]), A(params["fc_b"])
    wm["fcw"] = np.ascontiguousarray(fcW * bn_g[:, None])
    wm["fcb"] = np.ascontiguousarray((fcb + bn_b @ fcW).reshape(1, LAT))
    dec = params["dec"]
    wm["d1w"] = np.ascontiguousarray(A(dec[0]["W"]))
    wm["d1b"] = np.ascontiguousarray(A(dec[0]["b"]).reshape(HD, 1))
    wm["d2w"] = np.ascontiguousarray(A(dec[1]["W"]))
    wm["d2b"] = np.ascontiguousarray(A(dec[1]["b"]).reshape(HD, 1))
    wm["gsel"] = np.ascontiguousarray(
        np.tile(np.eye(CH, dtype=np.float32).reshape(1, CH * CH), (128, 1)))
    wm["onesr"] = np.ones((1, 128), dtype=np.float32)

    # padded final layer: column (i,j) = W3[:, ut_idx] (sym), 0 on diag
    W3, b3 = A(dec[2]["W"]), A(dec[2]["b"])
    iu0, iu1 = np.triu_indices(NPG, k=1)
    ut_of = np.zeros((NPG, NPG), dtype=np.int64)
    ut_of[iu0, iu1] = np.arange(N_UT)
    ut_of[iu1, iu0] = np.arange(N_UT)
    Wp = np.zeros((HD, NPG * NPG), dtype=np.float32)
    bp = np.full((NPG * NPG,), -10.0, dtype=np.float32)
    off = np.where(np.eye(NPG, dtype=bool).ravel(), -1, ut_of.ravel())
    nz = off >= 0
    Wp[:, nz] = W3[:, off[nz]]
    bp[nz] = b3[off[nz]]
    Wp = Wp.reshape(HD, NPG, NPG)
    bp = bp.reshape(NPG, NPG)

    in_maps = []
    for c in range(N_CORES):
        m = dict(wm)
        gs = c * GPC
        m["x"] = np.ascontiguousarray(x[gs * NPG:(gs + GPC) * NPG])
        m["at"] = np.ascontiguousarray(at[gs:gs + GPC])  # [g, s, d]
        r0 = c * ROWS_PC
        m["d3w"] = np.ascontiguousarray(
            Wp[:, r0:r0 + ROWS_PC, :].reshape(HD, NOUT))
        m["d3b"] = np.ascontiguousarray(
            bp[r0:r0 + ROWS_PC, :].reshape(1, NOUT))
        in_maps.append(m)
    return in_maps


def kernel(x, edge_index, batch, params):
    if "nc" not in _CACHE:
        _CACHE["nc"] = _build_program()
    nc = _CACHE["nc"]
    in_maps = _prep_host(x, edge_index, batch, params)

    from concourse.bass_utils import run_bass_kernel_spmd
    res = run_bass_kernel_spmd(nc, in_maps, list(range(N_CORES)))
    # core c holds adjacency rows [c*20, c*20+20) for all graphs
    slabs = [res.results[c]["out"].reshape(B, ROWS_PC, NPG)
             for c in range(N_CORES)]
    return np.ascontiguousarray(np.concatenate(slabs, axis=1))


# revision 15
# speedup vs baseline: 1.6048x; 1.3622x over previous
"""GIN autoencoder forward pass on 8 Trainium2 NeuronCores.

Strategy (data-parallel encoder over graphs, model-parallel decoder):
- Host: re-encode the (static-topology) edge list as dense per-graph
  adjacency AT[g] = A[g]^T + I (A[d,s] = multiplicity of edge s->d) so GIN
  aggregation z = (I+A) @ h becomes a block-diagonal matmul. BatchNorm (eval
  mode) is folded into the following Linear. The decoder's upper-triangle
  scatter + symmetrize + diag(-10) is folded into a padded final weight
  matrix W3p[:, i*160+j] = W3[:, ut_idx(min(i,j),max(i,j))] (0 on diag,
  bias -10 on diag), sharded by output rows across cores.
- Device: 32 graphs/core through 4 GIN layers (fp32r matmuls), sum-pool via
  ones-matmul, fc -> latent; decoder d1/d2 per-core, AllGather of z2, then
  each core computes adjacency rows [20c, 20c+20) for ALL 256 graphs.
"""
import sys

for _p in ("/opt/trn_rl_repo", "/root/.axon_site/_ro/trn_rl_repo"):
    if _p not in sys.path:
        sys.path.append(_p)

import numpy as np

import concourse.bacc as bacc
import concourse.bass as bass
import concourse.tile as tile
from concourse import mybir

F32 = mybir.dt.float32
F32R = mybir.dt.float32r
AF = mybir.ActivationFunctionType

# ---- problem geometry (hardcoded; matches reference.py) ----
B = 256                 # graphs
NPG = 160               # nodes per graph
DEG = 16
N_NODES = B * NPG
N_EDGES = N_NODES * DEG
F0 = 128                # input feature dim
FH = 256                # encoder hidden
LAT = 64
HD = 512
NL_ENC = 4
N_UT = NPG * (NPG - 1) // 2
BN_EPS = 1e-5
ALPHA = 0.2             # LeakyReLU slope

N_CORES = 8
GPC = B // N_CORES      # 32 graphs per core
CH = 16                 # graphs per chunk
NCHUNK = GPC // CH      # 2
ROWS_PC = NPG // N_CORES            # 20 adjacency rows per core
NOUT = ROWS_PC * NPG                # 3200 padded output cols per core
S_HI, S_LO = 128, NPG - 128         # node split within a graph
ATP = 256                           # AT padded cols (fp32r fast path)
SPAD = 256                          # AT padded rows (full K=128 row groups)

_CACHE = {}


def _build_program():
    nc = bacc.Bacc(target_bir_lowering=False)

    # ---------------- DRAM I/O ----------------
    x_d = nc.dram_tensor("x", [GPC * NPG, F0], F32, kind="ExternalInput")
    at_d = nc.dram_tensor("at", [GPC, SPAD, ATP], F32, kind="ExternalInput")
    w1_d, b1_d, w2_d, b2_d = [], [], [], []
    for l in range(NL_ENC):
        fin = F0 if l == 0 else FH
        w1_d.append(nc.dram_tensor(f"w1_{l}", [fin, FH], F32, kind="ExternalInput"))
        b1_d.append(nc.dram_tensor(f"b1_{l}", [FH, 1], F32, kind="ExternalInput"))
        w2_d.append(nc.dram_tensor(f"w2_{l}", [FH, FH], F32, kind="ExternalInput"))
        b2_d.append(nc.dram_tensor(f"b2_{l}", [1, FH], F32, kind="ExternalInput"))
    fcw_d = nc.dram_tensor("fcw", [FH, LAT], F32, kind="ExternalInput")
    fcb_d = nc.dram_tensor("fcb", [1, LAT], F32, kind="ExternalInput")
    d1w_d = nc.dram_tensor("d1w", [LAT, HD], F32, kind="ExternalInput")
    d1b_d = nc.dram_tensor("d1b", [HD, 1], F32, kind="ExternalInput")
    d2w_d = nc.dram_tensor("d2w", [HD, HD], F32, kind="ExternalInput")
    d2b_d = nc.dram_tensor("d2b", [HD, 1], F32, kind="ExternalInput")
    d3w_d = nc.dram_tensor("d3w", [HD, NOUT], F32, kind="ExternalInput")
    gsel_d = nc.dram_tensor("gsel", [SPAD, CH * CH], F32, kind="ExternalInput")
    onesr_d = nc.dram_tensor("onesr", [1, 128], F32, kind="ExternalInput")
    zeros_d = nc.dram_tensor("zeros", [1, CH * FH], F32, kind="ExternalInput")
    d3b_d = nc.dram_tensor("d3b", [1, NOUT], F32, kind="ExternalInput")
    out_d = nc.dram_tensor("out", [B, NOUT], F32, kind="ExternalOutput")
    # collective bounce buffers (collectives can't touch I/O tensors)
    cc_in = nc.dram_tensor("cc_in", [HD, GPC], F32)
    cc_out = nc.dram_tensor("cc_out", [N_CORES * HD, GPC], F32, addr_space="Shared")

    def r(ap):
        return ap.bitcast(F32R)

    from contextlib import ExitStack
    ctx = ExitStack()
    with tile.TileContext(nc) as tc, ctx:
        wpool = ctx.enter_context(tc.tile_pool(name="weights", bufs=1))
        hpool = ctx.enter_context(tc.tile_pool(name="acts_h", bufs=2))
        zpool = ctx.enter_context(tc.tile_pool(name="acts_z", bufs=1))
        atpool = ctx.enter_context(tc.tile_pool(name="at", bufs=1))
        d3pool = ctx.enter_context(tc.tile_pool(name="d3w", bufs=2))
        opool = ctx.enter_context(tc.tile_pool(name="outstage", bufs=2))
        psZ = ctx.enter_context(tc.tile_pool(name="psZ", bufs=2, space="PSUM"))
        psA = ctx.enter_context(tc.tile_pool(name="psA", bufs=2, space="PSUM"))
        psBhi = ctx.enter_context(tc.tile_pool(name="psBhi", bufs=2, space="PSUM"))
        psBlo = ctx.enter_context(tc.tile_pool(name="psBlo", bufs=1, space="PSUM"))
        psS = ctx.enter_context(tc.tile_pool(name="psS", bufs=1, space="PSUM"))

        # ---------------- static weights to SBUF ----------------
        w1_sb, b1_sb, w2_sb, b2_sb = [], [], [], []
        for l in range(NL_ENC):
            fin = F0 if l == 0 else FH
            nk = fin // 128
            t = wpool.tile([128, nk, FH], F32R, tag=f"w1_{l}")
            nc.sync.dma_start(out=t[:], in_=bass.AP(
                w1_d[l], 0, [[FH, 128], [128 * FH, nk], [1, FH]]).bitcast(F32R))
            w1_sb.append(t)
            tb = wpool.tile([128, FH // 128, 1], F32, tag=f"b1_{l}")
            nc.sync.dma_start(out=tb[:], in_=bass.AP(
                b1_d[l], 0, [[1, 128], [128, FH // 128], [1, 1]]))
            b1_sb.append(tb)
            t2 = wpool.tile([128, FH // 128, FH], F32R, tag=f"w2_{l}")
            nc.sync.dma_start(out=t2[:], in_=bass.AP(
                w2_d[l], 0, [[FH, 128], [128 * FH, FH // 128], [1, FH]]).bitcast(F32R))
            w2_sb.append(t2)
            t3 = wpool.tile([128, FH], F32, tag=f"b2_{l}")
            nc.sync.dma_start(out=t3[:], in_=bass.AP(
                b2_d[l], 0, [[0, 128], [1, FH]]))
            b2_sb.append(t3)
        fcw_sb = wpool.tile([128, FH // 128, LAT], F32R, tag="fcw")
        nc.sync.dma_start(out=fcw_sb[:], in_=bass.AP(
            fcw_d, 0, [[LAT, 128], [128 * LAT, FH // 128], [1, LAT]]).bitcast(F32R))
        fcb_sb = wpool.tile([1, LAT], F32R, tag="fcb")
        nc.sync.dma_start(out=fcb_sb[:], in_=r(fcb_d[:, :]))
        d1w_sb = wpool.tile([LAT, HD], F32R, tag="d1w")
        nc.sync.dma_start(out=d1w_sb[:], in_=r(d1w_d[:, :]))
        d1b_sb = wpool.tile([128, HD // 128, 1], F32, tag="d1b")
        nc.sync.dma_start(out=d1b_sb[:], in_=bass.AP(
            d1b_d, 0, [[1, 128], [128, HD // 128], [1, 1]]))
        d2w_sb = wpool.tile([128, HD // 128, HD], F32R, tag="d2w")
        nc.sync.dma_start(out=d2w_sb[:], in_=bass.AP(
            d2w_d, 0, [[HD, 128], [128 * HD, HD // 128], [1, HD]]).bitcast(F32R))
        d2b_sb = wpool.tile([128, HD // 128, 1], F32, tag="d2b")
        nc.sync.dma_start(out=d2b_sb[:], in_=bass.AP(
            d2b_d, 0, [[1, 128], [128, HD // 128], [1, 1]]))
        d3b_sb = wpool.tile([128, NOUT], F32, tag="d3b")
        nc.sync.dma_start(out=d3b_sb[:], in_=bass.AP(
            d3b_d, 0, [[0, 128], [1, NOUT]]))

        alpha_ap = wpool.tile([128, 1], F32, tag="alpha")
        nc.vector.memset(alpha_ap[:], ALPHA)
        gsel_sb = wpool.tile([128, 2, CH * CH], F32R, tag="gsel")
        nc.sync.dma_start(out=gsel_sb[:], in_=bass.AP(
            gsel_d, 0, [[CH * CH, 128], [128 * CH * CH, 2], [1, CH * CH]]
        ).bitcast(F32R))
        ones_row = wpool.tile([1, 128], F32R, tag="ones_row")
        nc.sync.dma_start(out=ones_row[:], in_=r(onesr_d[:, :]))

        pooledT = zpool.tile([128, FH // 128, GPC], F32R, tag="pooledT")
        h_loA = zpool.tile([128, CH, FH], F32R, tag="h_loA")
        h_loB = zpool.tile([128, CH, FH], F32R, tag="h_loB")
        h_lo_bufs = [h_loA, h_loB]
        for _hb in h_lo_bufs:
            for _p0 in range(S_LO, 128, 32):
                nc.sync.dma_start(out=_hb[_p0:_p0 + 32, :, :], in_=bass.AP(
                    zeros_d, 0, [[0, 32], [FH, CH], [1, FH]]).bitcast(F32R))

        # ---------------- encoder, per chunk of 16 graphs ----------------
        for ch in range(NCHUNK):
            g0 = ch * CH
            # load x chunk into layer-0 h buffers (node-major, graph-blocked)
            h_hi = hpool.tile([128, CH, FH], F32R, tag="h_hi")
            h_lo = h_lo_bufs[0]
            nc.sync.dma_start(out=h_hi[:, :, 0:F0], in_=bass.AP(
                x_d, g0 * NPG * F0,
                [[F0, 128], [NPG * F0, CH], [1, F0]]).bitcast(F32R))
            nc.sync.dma_start(out=h_lo[0:S_LO, :, 0:F0], in_=bass.AP(
                x_d, (g0 * NPG + S_HI) * F0,
                [[F0, S_LO], [NPG * F0, CH], [1, F0]]).bitcast(F32R))
            at_hi = atpool.tile([128, CH, ATP], F32R, tag="at_hi")
            at_lo = atpool.tile([128, CH, ATP], F32R, tag="at_lo")
            nc.sync.dma_start(out=at_hi[:], in_=bass.AP(
                at_d, g0 * SPAD * ATP,
                [[ATP, 128], [SPAD * ATP, CH], [1, ATP]]).bitcast(F32R))
            nc.sync.dma_start(out=at_lo[:], in_=bass.AP(
                at_d, g0 * SPAD * ATP + S_HI * ATP,
                [[ATP, 128], [SPAD * ATP, CH], [1, ATP]]).bitcast(F32R))

            for l in range(NL_ENC):
                fin = F0 if l == 0 else FH
                nm = fin // 128   # M tiles for zT / K tiles for MLP1
                # ---- zT = h^T AT^ (aggregation incl. +I), feature-major ----
                zT = zpool.tile([128, FH // 128, CH * NPG], F32R, tag="zT")
                for g in range(0, CH, 2):
                    for m in range(nm):
                        pz = psZ.tile([128, 2, ATP], F32, tag="psZ")
                        for j in (0, 1):
                            gg = g + j
                            nc.tensor.matmul(
                                pz[:, j, :],
                                lhsT=h_hi[:, gg, m * 128:(m + 1) * 128],
                                rhs=at_hi[:, gg, :], start=True, stop=False)
                            nc.tensor.matmul(
                                pz[:, j, :],
                                lhsT=h_lo[:, gg, m * 128:(m + 1) * 128],
                                rhs=at_lo[:, gg, :], start=False, stop=True)
                        nc.vector.tensor_copy(
                            zT[:, m, g * NPG:(g + 2) * NPG].rearrange(
                                "p (two c) -> p two c", two=2),
                            pz[:, :, 0:NPG])
                # ---- y1T = Prelu(W1^T z + b1), feature-major ----
                y1T = zpool.tile([128, FH // 128, CH * NPG], F32R, tag="y1T")
                NW = 512
                nn = (CH * NPG) // NW  # 5
                for m in range(FH // 128):
                    for n in range(nn):
                        pa = psA.tile([128, NW], F32, tag="psA")
                        for k in range(nm):
                            nc.tensor.matmul(
                                pa[:], lhsT=w1_sb[l][:, k, m * 128:(m + 1) * 128],
                                rhs=zT[:, k, n * NW:(n + 1) * NW],
                                start=(k == 0), stop=(k == nm - 1))
                        nc.scalar.activation(
                            y1T[:, m, n * NW:(n + 1) * NW], pa[:],
                            AF.Prelu, bias=b1_sb[l][:, m, 0:1], scale=1.0,
                            alpha=alpha_ap[:])
                # ---- h_next = Prelu(y1 W2 + b2), node-major (per graph) ----
                hn_hi = hpool.tile([128, CH, FH], F32R, tag="h_hi")
                hn_lo = h_lo_bufs[(l + 1) % 2]
                for g in range(CH):
                    c0 = g * NPG
                    pb = psBhi.tile([128, FH], F32, tag="psB_hi")
                    for k in range(FH // 128):
                        nc.tensor.matmul(
                            pb[:], lhsT=y1T[:, k, c0:c0 + 128],
                            rhs=w2_sb[l][:, k, :], start=(k == 0),
                            stop=(k == FH // 128 - 1))
                    nc.vector.tensor_tensor(
                        out=hn_hi[:, g, :], in0=pb[:],
                        in1=b2_sb[l][:],
                        op=mybir.AluOpType.add)
                    nc.scalar.activation(hn_hi[:, g, :], hn_hi[:, g, :],
                                         AF.Prelu, alpha=alpha_ap[:])
                    pc = psBlo.tile([S_LO, FH], F32, tag="psB_lo")
                    for k in range(FH // 128):
                        nc.tensor.matmul(
                            pc[:], lhsT=y1T[:, k, c0 + S_HI:c0 + NPG],
                            rhs=w2_sb[l][:, k, :], start=(k == 0),
                            stop=(k == FH // 128 - 1))
                    nc.vector.tensor_tensor(
                        out=hn_lo[0:S_LO, g, :], in0=pc[:],
                        in1=b2_sb[l][0:S_LO, :],
                        op=mybir.AluOpType.add)
                    nc.scalar.activation(hn_lo[0:S_LO, g, :],
                                         hn_lo[0:S_LO, g, :],
                                         AF.Prelu, alpha=alpha_ap[0:S_LO, :])
                h_hi, h_lo = hn_hi, hn_lo

            # ---- sum-pool h4 over nodes (per graph) -> pooledT cols ----
            for m in range(FH // 128):
                pp = psS.tile([128, CH], F32, tag="psSmall")
                for g in range(CH):
                    nc.tensor.matmul(
                        pp[:], lhsT=h_hi[:, g, m * 128:(m + 1) * 128],
                        rhs=gsel_sb[:, 0, g * CH:(g + 1) * CH],
                        start=(g == 0), stop=False)
                    nc.tensor.matmul(
                        pp[:], lhsT=h_lo[:, g, m * 128:(m + 1) * 128],
                        rhs=gsel_sb[:, 1, g * CH:(g + 1) * CH],
                        start=False, stop=(g == CH - 1))
                nc.vector.tensor_copy(pooledT[:, m, g0:g0 + CH], pp[:])

        # ---------------- latent + decoder d1/d2 (per-core graphs) ----------
        latT = zpool.tile([LAT, GPC], F32R, tag="latT")
        pl = psS.tile([LAT, GPC], F32, tag="psSmall")
        for k in range(FH // 128):
            nc.tensor.matmul(pl[:], lhsT=fcw_sb[:, k, :], rhs=pooledT[:, k, :],
                             start=(k == 0), stop=False)
        nc.tensor.matmul(pl[:], lhsT=fcb_sb[:], rhs=ones_row[:, 0:GPC],
                         start=False, stop=True)
        nc.vector.tensor_copy(latT[:], pl[:])

        ydT = zpool.tile([128, HD // 128, GPC], F32R, tag="ydT")
        for m in range(HD // 128):
            pd = psS.tile([128, GPC], F32, tag="psSmall")
            nc.tensor.matmul(pd[:], lhsT=d1w_sb[:, m * 128:(m + 1) * 128],
                             rhs=latT[:], start=True, stop=True)
            nc.scalar.activation(ydT[:, m, :], pd[:], AF.Relu,
                                 bias=d1b_sb[:, m, 0:1])
        z2T = zpool.tile([128, HD // 128, GPC], F32, tag="z2T")
        for m in range(HD // 128):
            pd = psS.tile([128, GPC], F32, tag="psSmall")
            for k in range(HD // 128):
                nc.tensor.matmul(pd[:], lhsT=d2w_sb[:, k, m * 128:(m + 1) * 128],
                                 rhs=ydT[:, k, :],
                                 start=(k == 0), stop=(k == HD // 128 - 1))
            nc.scalar.activation(z2T[:, m, :], pd[:], AF.Relu,
                                 bias=d2b_sb[:, m, 0:1])

        # ---------------- all-gather z2 across cores ----------------
        nc.sync.dma_start(out=bass.AP(
            cc_in, 0, [[GPC, 128], [128 * GPC, HD // 128], [1, GPC]]),
            in_=z2T[:])
        nc.gpsimd.collective_compute(
            "AllGather", mybir.AluOpType.bypass,
            replica_groups=[list(range(N_CORES))],
            ins=[cc_in.ap().opt()], outs=[cc_out.ap().opt()])
        z2all = zpool.tile([128, HD // 128, B], F32R, tag="z2all")
        for c in range(N_CORES):
            for k in range(HD // 128):
                nc.sync.dma_start(
                    out=z2all[:, k, c * GPC:(c + 1) * GPC],
                    in_=bass.AP(cc_out, (c * HD + k * 128) * GPC,
                                [[GPC, 128], [1, GPC]]).bitcast(F32R))

        # ---------------- model-parallel d3: adjacency row-slab ----------
        NW3 = 512
        n_n3 = (NOUT + NW3 - 1) // NW3
        for n in range(n_n3):
            c0 = n * NW3
            cw = min(NW3, NOUT - c0)
            w3t = d3pool.tile([128, HD // 128, NW3], F32R, tag="w3")
            nc.sync.dma_start(out=w3t[:, :, 0:cw], in_=bass.AP(
                d3w_d, c0, [[NOUT, 128], [128 * NOUT, HD // 128], [1, cw]]
            ).bitcast(F32R))
            for m in range(B // 128):
                po = psA.tile([128, NW3], F32, tag="psA")
                for k in range(HD // 128):
                    nc.tensor.matmul(
                        po[:, 0:cw], lhsT=z2all[:, k, m * 128:(m + 1) * 128],
                        rhs=w3t[:, k, 0:cw],
                        start=(k == 0), stop=(k == HD // 128 - 1))
                ot = opool.tile([128, NW3], F32, tag="ostage")
                nc.vector.tensor_tensor(
                    out=ot[:, 0:cw], in0=po[:, 0:cw],
                    in1=d3b_sb[:, c0:c0 + cw],
                    op=mybir.AluOpType.add)
                nc.sync.dma_start(
                    out=out_d[m * 128:(m + 1) * 128, c0:c0 + cw],
                    in_=ot[:, 0:cw])

    nc.compile()
    return nc


def _prep_host(x, edge_index, batch, params):
    """Shard inputs + fold BN / build dense AT / pad decoder weights."""
    x = np.asarray(x, dtype=np.float32)
    src = np.asarray(edge_index[0], dtype=np.int64)
    dst = np.asarray(edge_index[1], dtype=np.int64)

    # dense AT[g, s, d] = #edges s->d in graph g, + I (GIN eps=0 self term)
    g_of_e = src // NPG
    code = (g_of_e * NPG + (src % NPG)) * NPG + (dst % NPG)
    at = np.bincount(code, minlength=B * NPG * NPG).reshape(B, NPG, NPG)
    at = at.astype(np.float32)
    at += np.eye(NPG, dtype=np.float32)[None]
    at = np.concatenate(
        [at, np.zeros((B, NPG, ATP - NPG), np.float32)], axis=2)
    at = np.concatenate(
        [at, np.zeros((B, SPAD - NPG, ATP), np.float32)], axis=1)

    inv_std = np.float32(1.0 / np.sqrt(np.float32(1.0 + BN_EPS)))

    def A(v):
        return np.asarray(v, dtype=np.float32)

    wm = {}
    for l, lyr in enumerate(params["enc"]):
        W1, b1 = A(lyr["W1"]), A(lyr["b1"])
        g, bt = A(lyr["g"]) * inv_std, A(lyr["bt"])
        W2, b2 = A(lyr["W2"]), A(lyr["b2"])
        wm[f"w1_{l}"] = np.ascontiguousarray(W1)
        wm[f"b1_{l}"] = np.ascontiguousarray(b1.reshape(FH, 1))
        wm[f"w2_{l}"] = np.ascontiguousarray(W2 * g[:, None])
        wm[f"b2_{l}"] = np.ascontiguousarray((b2 + bt @ W2).reshape(1, FH))
    bn_g = A(params["bn_g"]) * inv_std
    bn_b = A(params["bn_b"])
    fcW, fcb = A(params["fc_W"]), A(params["fc_b"])
    wm["fcw"] = np.ascontiguousarray(fcW * bn_g[:, None])
    wm["fcb"] = np.ascontiguousarray((fcb + bn_b @ fcW).reshape(1, LAT))
    dec = params["dec"]
    wm["d1w"] = np.ascontiguousarray(A(dec[0]["W"]))
    wm["d1b"] = np.ascontiguousarray(A(dec[0]["b"]).reshape(HD, 1))
    wm["d2w"] = np.ascontiguousarray(A(dec[1]["W"]))
    wm["d2b"] = np.ascontiguousarray(A(dec[1]["b"]).reshape(HD, 1))
    gs = np.tile(np.eye(CH, dtype=np.float32).reshape(1, CH * CH), (SPAD, 1))
    gs[NPG:SPAD] = 0.0   # zero rows for padded nodes in lo tile
    wm["gsel"] = np.ascontiguousarray(gs)
    wm["onesr"] = np.ones((1, 128), dtype=np.float32)
    wm["zeros"] = np.zeros((1, CH * FH), dtype=np.float32)

    # padded final layer: column (i,j) = W3[:, ut_idx] (sym), 0 on diag
    W3, b3 = A(dec[2]["W"]), A(dec[2]["b"])
    iu0, iu1 = np.triu_indices(NPG, k=1)
    ut_of = np.zeros((NPG, NPG), dtype=np.int64)
    ut_of[iu0, iu1] = np.arange(N_UT)
    ut_of[iu1, iu0] = np.arange(N_UT)
    Wp = np.zeros((HD, NPG * NPG), dtype=np.float32)
    bp = np.full((NPG * NPG,), -10.0, dtype=np.float32)
    off = np.where(np.eye(NPG, dtype=bool).ravel(), -1, ut_of.ravel())
    nz = off >= 0
    Wp[:, nz] = W3[:, off[nz]]
    bp[nz] = b3[off[nz]]
    Wp = Wp.reshape(HD, NPG, NPG)
    bp = bp.reshape(NPG, NPG)

    in_maps = []
    for c in range(N_CORES):
        m = dict(wm)
        gs = c * GPC
        m["x"] = np.ascontiguousarray(x[gs * NPG:(gs + GPC) * NPG])
        m["at"] = np.ascontiguousarray(at[gs:gs + GPC])  # [g, s, d]
        r0 = c * ROWS_PC
        m["d3w"] = np.ascontiguousarray(
            Wp[:, r0:r0 + ROWS_PC, :].reshape(HD, NOUT))
        m["d3b"] = np.ascontiguousarray(
            bp[r0:r0 + ROWS_PC, :].reshape(1, NOUT))
        in_maps.append(m)
    return in_maps


def kernel(x, edge_index, batch, params):
    if "nc" not in _CACHE:
        _CACHE["nc"] = _build_program()
    nc = _CACHE["nc"]
    in_maps = _prep_host(x, edge_index, batch, params)

    from concourse.bass_utils import run_bass_kernel_spmd
    res = run_bass_kernel_spmd(nc, in_maps, list(range(N_CORES)))
    # core c holds adjacency rows [c*20, c*20+20) for all graphs
    slabs = [res.results[c]["out"].reshape(B, ROWS_PC, NPG)
             for c in range(N_CORES)]
    return np.ascontiguousarray(np.concatenate(slabs, axis=1))


# revision 16
# speedup vs baseline: 1.7825x; 1.1107x over previous
"""GIN autoencoder forward pass on 8 Trainium2 NeuronCores.

Strategy (data-parallel encoder over graphs, model-parallel decoder):
- Host: re-encode the (static-topology) edge list as dense per-graph
  adjacency AT[g] = A[g]^T + I (A[d,s] = multiplicity of edge s->d) so GIN
  aggregation z = (I+A) @ h becomes a block-diagonal matmul. BatchNorm (eval
  mode) is folded into the following Linear. The decoder's upper-triangle
  scatter + symmetrize + diag(-10) is folded into a padded final weight
  matrix W3p[:, i*160+j] = W3[:, ut_idx(min(i,j),max(i,j))] (0 on diag,
  bias -10 on diag), sharded by output rows across cores.
- Device: 32 graphs/core through 4 GIN layers (fp32r matmuls), sum-pool via
  ones-matmul, fc -> latent; decoder d1/d2 per-core, AllGather of z2, then
  each core computes adjacency rows [20c, 20c+20) for ALL 256 graphs.
"""
import sys

for _p in ("/opt/trn_rl_repo", "/root/.axon_site/_ro/trn_rl_repo"):
    if _p not in sys.path:
        sys.path.append(_p)

import numpy as np

import concourse.bacc as bacc
import concourse.bass as bass
import concourse.tile as tile
from concourse import mybir

F32 = mybir.dt.float32
F32R = mybir.dt.float32r
AF = mybir.ActivationFunctionType

# ---- problem geometry (hardcoded; matches reference.py) ----
B = 256                 # graphs
NPG = 160               # nodes per graph
DEG = 16
N_NODES = B * NPG
N_EDGES = N_NODES * DEG
F0 = 128                # input feature dim
FH = 256                # encoder hidden
LAT = 64
HD = 512
NL_ENC = 4
N_UT = NPG * (NPG - 1) // 2
BN_EPS = 1e-5
ALPHA = 0.2             # LeakyReLU slope

N_CORES = 8
GPC = B // N_CORES      # 32 graphs per core
CH = 16                 # graphs per chunk
NCHUNK = GPC // CH      # 2
ROWS_PC = NPG // N_CORES            # 20 adjacency rows per core
NOUT = ROWS_PC * NPG                # 3200 padded output cols per core
S_HI, S_LO = 128, NPG - 128         # node split within a graph
ATP = 256                           # AT padded cols (fp32r fast path)
SPAD = 256                          # AT padded rows (full K=128 row groups)

_CACHE = {}


def _build_program():
    nc = bacc.Bacc(target_bir_lowering=False)

    # ---------------- DRAM I/O ----------------
    x_d = nc.dram_tensor("x", [GPC * NPG, F0], F32, kind="ExternalInput")
    at_d = nc.dram_tensor("at", [GPC, SPAD, ATP], F32, kind="ExternalInput")
    w1_d, b1_d, w2_d, b2_d = [], [], [], []
    for l in range(NL_ENC):
        fin = F0 if l == 0 else FH
        w1_d.append(nc.dram_tensor(f"w1_{l}", [fin, FH], F32, kind="ExternalInput"))
        b1_d.append(nc.dram_tensor(f"b1_{l}", [FH, 1], F32, kind="ExternalInput"))
        w2_d.append(nc.dram_tensor(f"w2_{l}", [FH, FH], F32, kind="ExternalInput"))
        b2_d.append(nc.dram_tensor(f"b2_{l}", [1, FH], F32, kind="ExternalInput"))
    fcw_d = nc.dram_tensor("fcw", [FH, LAT], F32, kind="ExternalInput")
    fcb_d = nc.dram_tensor("fcb", [1, LAT], F32, kind="ExternalInput")
    d1w_d = nc.dram_tensor("d1w", [LAT, HD], F32, kind="ExternalInput")
    d1b_d = nc.dram_tensor("d1b", [HD, 1], F32, kind="ExternalInput")
    d2w_d = nc.dram_tensor("d2w", [HD, HD], F32, kind="ExternalInput")
    d2b_d = nc.dram_tensor("d2b", [HD, 1], F32, kind="ExternalInput")
    d3w_d = nc.dram_tensor("d3w", [HD, NOUT], F32, kind="ExternalInput")
    gsel_d = nc.dram_tensor("gsel", [SPAD, CH * CH], F32, kind="ExternalInput")
    onesr_d = nc.dram_tensor("onesr", [1, 128], F32, kind="ExternalInput")
    zeros_d = nc.dram_tensor("zeros", [1, CH * FH], F32, kind="ExternalInput")
    d3b_d = nc.dram_tensor("d3b", [1, NOUT], F32, kind="ExternalInput")
    out_d = nc.dram_tensor("out", [B, NOUT], F32, kind="ExternalOutput")
    # collective bounce buffers (collectives can't touch I/O tensors)
    cc_in = nc.dram_tensor("cc_in", [HD, GPC], F32)
    cc_out = nc.dram_tensor("cc_out", [N_CORES * HD, GPC], F32, addr_space="Shared")

    def r(ap):
        return ap.bitcast(F32R)

    from contextlib import ExitStack
    ctx = ExitStack()
    with tile.TileContext(nc) as tc, ctx:
        wpool = ctx.enter_context(tc.tile_pool(name="weights", bufs=1))
        hpool = ctx.enter_context(tc.tile_pool(name="acts_h", bufs=2))
        zpool = ctx.enter_context(tc.tile_pool(name="acts_z", bufs=1))
        atpool = ctx.enter_context(tc.tile_pool(name="at", bufs=1))
        d3pool = ctx.enter_context(tc.tile_pool(name="d3w", bufs=2))
        opool = ctx.enter_context(tc.tile_pool(name="outstage", bufs=2))
        psZ = ctx.enter_context(tc.tile_pool(name="psZ", bufs=2, space="PSUM"))
        psA = ctx.enter_context(tc.tile_pool(name="psA", bufs=2, space="PSUM"))
        psBhi = ctx.enter_context(tc.tile_pool(name="psBhi", bufs=2, space="PSUM"))
        psBlo = ctx.enter_context(tc.tile_pool(name="psBlo", bufs=1, space="PSUM"))
        psS = ctx.enter_context(tc.tile_pool(name="psS", bufs=1, space="PSUM"))

        pooledT = zpool.tile([128, FH // 128, GPC], F32R, tag="pooledT")
        h_loA = zpool.tile([128, CH, FH], F32R, tag="h_loA")
        h_loB = zpool.tile([128, CH, FH], F32R, tag="h_loB")
        h_lo_bufs = [h_loA, h_loB]
        for _hb in h_lo_bufs:
            for _p0 in range(S_LO, 128, 32):
                nc.sync.dma_start(out=_hb[_p0:_p0 + 32, :, :], in_=bass.AP(
                    zeros_d, 0, [[0, 32], [FH, CH], [1, FH]]).bitcast(F32R))

        # ---- chunk input loader (issued before weights: encoder's first
        # graphs must not queue behind megabytes of decoder weights) ----
        hpool_tiles = {}

        def load_chunk(ch):
            g0 = ch * CH
            h_hi = hpool.tile([128, CH, FH], F32R, tag="h_hi", name=f"h_hi{ch}")
            h_lo = h_lo_bufs[0]
            nc.sync.dma_start(out=h_hi[:, :, 0:F0], in_=bass.AP(
                x_d, g0 * NPG * F0,
                [[F0, 128], [NPG * F0, CH], [1, F0]]).bitcast(F32R))
            nc.sync.dma_start(out=h_lo[0:S_LO, :, 0:F0], in_=bass.AP(
                x_d, (g0 * NPG + S_HI) * F0,
                [[F0, S_LO], [NPG * F0, CH], [1, F0]]).bitcast(F32R))
            at_hi = atpool.tile([128, CH, ATP], F32R, tag="at_hi",
                                name=f"at_hi{ch}")
            at_lo = atpool.tile([128, CH, ATP], F32R, tag="at_lo",
                                name=f"at_lo{ch}")
            nc.sync.dma_start(out=at_hi[:], in_=bass.AP(
                at_d, g0 * SPAD * ATP,
                [[ATP, 128], [SPAD * ATP, CH], [1, ATP]]).bitcast(F32R))
            nc.sync.dma_start(out=at_lo[:], in_=bass.AP(
                at_d, g0 * SPAD * ATP + S_HI * ATP,
                [[ATP, 128], [SPAD * ATP, CH], [1, ATP]]).bitcast(F32R))
            return h_hi, h_lo, at_hi, at_lo

        hpool_tiles[0] = load_chunk(0)

        # ---------------- static weights to SBUF ----------------
        w1_sb, b1_sb, w2_sb, b2_sb = [], [], [], []
        for l in range(NL_ENC):
            fin = F0 if l == 0 else FH
            nk = fin // 128
            t = wpool.tile([128, nk, FH], F32R, tag=f"w1_{l}")
            nc.sync.dma_start(out=t[:], in_=bass.AP(
                w1_d[l], 0, [[FH, 128], [128 * FH, nk], [1, FH]]).bitcast(F32R))
            w1_sb.append(t)
            tb = wpool.tile([128, FH // 128, 1], F32, tag=f"b1_{l}")
            nc.sync.dma_start(out=tb[:], in_=bass.AP(
                b1_d[l], 0, [[1, 128], [128, FH // 128], [1, 1]]))
            b1_sb.append(tb)
            t2 = wpool.tile([128, FH // 128, FH], F32R, tag=f"w2_{l}")
            nc.sync.dma_start(out=t2[:], in_=bass.AP(
                w2_d[l], 0, [[FH, 128], [128 * FH, FH // 128], [1, FH]]).bitcast(F32R))
            w2_sb.append(t2)
            t3 = wpool.tile([128, FH], F32, tag=f"b2_{l}")
            nc.sync.dma_start(out=t3[:], in_=bass.AP(
                b2_d[l], 0, [[0, 128], [1, FH]]))
            b2_sb.append(t3)
        fcw_sb = wpool.tile([128, FH // 128, LAT], F32R, tag="fcw")
        nc.sync.dma_start(out=fcw_sb[:], in_=bass.AP(
            fcw_d, 0, [[LAT, 128], [128 * LAT, FH // 128], [1, LAT]]).bitcast(F32R))
        fcb_sb = wpool.tile([1, LAT], F32R, tag="fcb")
        nc.sync.dma_start(out=fcb_sb[:], in_=r(fcb_d[:, :]))
        d1w_sb = wpool.tile([LAT, HD], F32R, tag="d1w")
        nc.sync.dma_start(out=d1w_sb[:], in_=r(d1w_d[:, :]))
        d1b_sb = wpool.tile([128, HD // 128, 1], F32, tag="d1b")
        nc.sync.dma_start(out=d1b_sb[:], in_=bass.AP(
            d1b_d, 0, [[1, 128], [128, HD // 128], [1, 1]]))
        d2w_sb = wpool.tile([128, HD // 128, HD], F32R, tag="d2w")
        nc.sync.dma_start(out=d2w_sb[:], in_=bass.AP(
            d2w_d, 0, [[HD, 128], [128 * HD, HD // 128], [1, HD]]).bitcast(F32R))
        d2b_sb = wpool.tile([128, HD // 128, 1], F32, tag="d2b")
        nc.sync.dma_start(out=d2b_sb[:], in_=bass.AP(
            d2b_d, 0, [[1, 128], [128, HD // 128], [1, 1]]))
        d3b_sb = wpool.tile([128, NOUT], F32, tag="d3b")
        nc.sync.dma_start(out=d3b_sb[:], in_=bass.AP(
            d3b_d, 0, [[0, 128], [1, NOUT]]))

        alpha_ap = wpool.tile([128, 1], F32, tag="alpha")
        nc.vector.memset(alpha_ap[:], ALPHA)
        gsel_sb = wpool.tile([128, 2, CH * CH], F32R, tag="gsel")
        nc.sync.dma_start(out=gsel_sb[:], in_=bass.AP(
            gsel_d, 0, [[CH * CH, 128], [128 * CH * CH, 2], [1, CH * CH]]
        ).bitcast(F32R))
        ones_row = wpool.tile([1, 128], F32R, tag="ones_row")
        nc.sync.dma_start(out=ones_row[:], in_=r(onesr_d[:, :]))


        # ---------------- encoder, per chunk of 16 graphs ----------------
        for ch in range(NCHUNK):
            g0 = ch * CH
            if ch not in hpool_tiles:
                hpool_tiles[ch] = load_chunk(ch)
            h_hi, h_lo, at_hi, at_lo = hpool_tiles[ch]

            for l in range(NL_ENC):
                fin = F0 if l == 0 else FH
                nm = fin // 128   # M tiles for zT / K tiles for MLP1
                # ---- zT = h^T AT^ (aggregation incl. +I), feature-major ----
                zT = zpool.tile([128, FH // 128, CH * NPG], F32R, tag="zT")
                for g in range(0, CH, 2):
                    for m in range(nm):
                        pz = psZ.tile([128, 2, ATP], F32, tag="psZ")
                        for j in (0, 1):
                            gg = g + j
                            nc.tensor.matmul(
                                pz[:, j, :],
                                lhsT=h_hi[:, gg, m * 128:(m + 1) * 128],
                                rhs=at_hi[:, gg, :], start=True, stop=False)
                            nc.tensor.matmul(
                                pz[:, j, :],
                                lhsT=h_lo[:, gg, m * 128:(m + 1) * 128],
                                rhs=at_lo[:, gg, :], start=False, stop=True)
                        nc.vector.tensor_copy(
                            zT[:, m, g * NPG:(g + 2) * NPG].rearrange(
                                "p (two c) -> p two c", two=2),
                            pz[:, :, 0:NPG])
                # ---- y1T = Prelu(W1^T z + b1), feature-major ----
                y1T = zpool.tile([128, FH // 128, CH * NPG], F32R, tag="y1T")
                NW = 512
                nn = (CH * NPG) // NW  # 5
                for m in range(FH // 128):
                    for n in range(nn):
                        pa = psA.tile([128, NW], F32, tag="psA")
                        for k in range(nm):
                            nc.tensor.matmul(
                                pa[:], lhsT=w1_sb[l][:, k, m * 128:(m + 1) * 128],
                                rhs=zT[:, k, n * NW:(n + 1) * NW],
                                start=(k == 0), stop=(k == nm - 1))
                        nc.scalar.activation(
                            y1T[:, m, n * NW:(n + 1) * NW], pa[:],
                            AF.Prelu, bias=b1_sb[l][:, m, 0:1], scale=1.0,
                            alpha=alpha_ap[:])
                # ---- h_next = Prelu(y1 W2 + b2), node-major (per graph) ----
                hn_hi = hpool.tile([128, CH, FH], F32R, tag="h_hi")
                hn_lo = h_lo_bufs[(l + 1) % 2]
                for g in range(CH):
                    c0 = g * NPG
                    pb = psBhi.tile([128, FH], F32, tag="psB_hi")
                    for k in range(FH // 128):
                        nc.tensor.matmul(
                            pb[:], lhsT=y1T[:, k, c0:c0 + 128],
                            rhs=w2_sb[l][:, k, :], start=(k == 0),
                            stop=(k == FH // 128 - 1))
                    nc.vector.tensor_tensor(
                        out=hn_hi[:, g, :], in0=pb[:],
                        in1=b2_sb[l][:],
                        op=mybir.AluOpType.add)
                    nc.scalar.activation(hn_hi[:, g, :], hn_hi[:, g, :],
                                         AF.Prelu, alpha=alpha_ap[:])
                    pc = psBlo.tile([S_LO, FH], F32, tag="psB_lo")
                    for k in range(FH // 128):
                        nc.tensor.matmul(
                            pc[:], lhsT=y1T[:, k, c0 + S_HI:c0 + NPG],
                            rhs=w2_sb[l][:, k, :], start=(k == 0),
                            stop=(k == FH // 128 - 1))
                    nc.vector.tensor_tensor(
                        out=hn_lo[0:S_LO, g, :], in0=pc[:],
                        in1=b2_sb[l][0:S_LO, :],
                        op=mybir.AluOpType.add)
                    nc.scalar.activation(hn_lo[0:S_LO, g, :],
                                         hn_lo[0:S_LO, g, :],
                                         AF.Prelu, alpha=alpha_ap[0:S_LO, :])
                h_hi, h_lo = hn_hi, hn_lo

            # ---- sum-pool h4 over nodes (per graph) -> pooledT cols ----
            for m in range(FH // 128):
                pp = psS.tile([128, CH], F32, tag="psSmall")
                for g in range(CH):
                    nc.tensor.matmul(
                        pp[:], lhsT=h_hi[:, g, m * 128:(m + 1) * 128],
                        rhs=gsel_sb[:, 0, g * CH:(g + 1) * CH],
                        start=(g == 0), stop=False)
                    nc.tensor.matmul(
                        pp[:], lhsT=h_lo[:, g, m * 128:(m + 1) * 128],
                        rhs=gsel_sb[:, 1, g * CH:(g + 1) * CH],
                        start=False, stop=(g == CH - 1))
                nc.vector.tensor_copy(pooledT[:, m, g0:g0 + CH], pp[:])

        # ---------------- latent + decoder d1/d2 (per-core graphs) ----------
        latT = zpool.tile([LAT, GPC], F32R, tag="latT")
        pl = psS.tile([LAT, GPC], F32, tag="psSmall")
        for k in range(FH // 128):
            nc.tensor.matmul(pl[:], lhsT=fcw_sb[:, k, :], rhs=pooledT[:, k, :],
                             start=(k == 0), stop=False)
        nc.tensor.matmul(pl[:], lhsT=fcb_sb[:], rhs=ones_row[:, 0:GPC],
                         start=False, stop=True)
        nc.vector.tensor_copy(latT[:], pl[:])

        ydT = zpool.tile([128, HD // 128, GPC], F32R, tag="ydT")
        for m in range(HD // 128):
            pd = psS.tile([128, GPC], F32, tag="psSmall")
            nc.tensor.matmul(pd[:], lhsT=d1w_sb[:, m * 128:(m + 1) * 128],
                             rhs=latT[:], start=True, stop=True)
            nc.scalar.activation(ydT[:, m, :], pd[:], AF.Relu,
                                 bias=d1b_sb[:, m, 0:1])
        z2T = zpool.tile([128, HD // 128, GPC], F32, tag="z2T")
        for m in range(HD // 128):
            pd = psS.tile([128, GPC], F32, tag="psSmall")
            for k in range(HD // 128):
                nc.tensor.matmul(pd[:], lhsT=d2w_sb[:, k, m * 128:(m + 1) * 128],
                                 rhs=ydT[:, k, :],
                                 start=(k == 0), stop=(k == HD // 128 - 1))
            nc.scalar.activation(z2T[:, m, :], pd[:], AF.Relu,
                                 bias=d2b_sb[:, m, 0:1])

        # ---------------- all-gather z2 across cores ----------------
        nc.sync.dma_start(out=bass.AP(
            cc_in, 0, [[GPC, 128], [128 * GPC, HD // 128], [1, GPC]]),
            in_=z2T[:])
        nc.gpsimd.collective_compute(
            "AllGather", mybir.AluOpType.bypass,
            replica_groups=[list(range(N_CORES))],
            ins=[cc_in.ap().opt()], outs=[cc_out.ap().opt()])
        z2all = zpool.tile([128, HD // 128, B], F32R, tag="z2all")
        for c in range(N_CORES):
            nc.sync.dma_start(
                out=z2all[:, :, c * GPC:(c + 1) * GPC],
                in_=bass.AP(cc_out, c * HD * GPC,
                            [[GPC, 128], [128 * GPC, HD // 128], [1, GPC]]
                            ).bitcast(F32R))

        # ---------------- model-parallel d3: adjacency row-slab ----------
        NW3 = 512
        n_n3 = (NOUT + NW3 - 1) // NW3
        for n in range(n_n3):
            c0 = n * NW3
            cw = min(NW3, NOUT - c0)
            w3t = d3pool.tile([128, HD // 128, NW3], F32R, tag="w3")
            nc.sync.dma_start(out=w3t[:, :, 0:cw], in_=bass.AP(
                d3w_d, c0, [[NOUT, 128], [128 * NOUT, HD // 128], [1, cw]]
            ).bitcast(F32R))
            for m in range(B // 128):
                po = psA.tile([128, NW3], F32, tag="psA")
                for k in range(HD // 128):
                    nc.tensor.matmul(
                        po[:, 0:cw], lhsT=z2all[:, k, m * 128:(m + 1) * 128],
                        rhs=w3t[:, k, 0:cw],
                        start=(k == 0), stop=(k == HD // 128 - 1))
                ot = opool.tile([128, NW3], F32, tag="ostage")
                nc.vector.tensor_tensor(
                    out=ot[:, 0:cw], in0=po[:, 0:cw],
                    in1=d3b_sb[:, c0:c0 + cw],
                    op=mybir.AluOpType.add)
                nc.sync.dma_start(
                    out=out_d[m * 128:(m + 1) * 128, c0:c0 + cw],
                    in_=ot[:, 0:cw])

    nc.compile()
    return nc


def _prep_host(x, edge_index, batch, params):
    """Shard inputs + fold BN / build dense AT / pad decoder weights."""
    x = np.asarray(x, dtype=np.float32)
    src = np.asarray(edge_index[0], dtype=np.int64)
    dst = np.asarray(edge_index[1], dtype=np.int64)

    # dense AT[g, s, d] = #edges s->d in graph g, + I (GIN eps=0 self term)
    g_of_e = src // NPG
    code = (g_of_e * NPG + (src % NPG)) * NPG + (dst % NPG)
    at = np.bincount(code, minlength=B * NPG * NPG).reshape(B, NPG, NPG)
    at = at.astype(np.float32)
    at += np.eye(NPG, dtype=np.float32)[None]
    at = np.concatenate(
        [at, np.zeros((B, NPG, ATP - NPG), np.float32)], axis=2)
    at = np.concatenate(
        [at, np.zeros((B, SPAD - NPG, ATP), np.float32)], axis=1)

    inv_std = np.float32(1.0 / np.sqrt(np.float32(1.0 + BN_EPS)))

    def A(v):
        return np.asarray(v, dtype=np.float32)

    wm = {}
    for l, lyr in enumerate(params["enc"]):
        W1, b1 = A(lyr["W1"]), A(lyr["b1"])
        g, bt = A(lyr["g"]) * inv_std, A(lyr["bt"])
        W2, b2 = A(lyr["W2"]), A(lyr["b2"])
        wm[f"w1_{l}"] = np.ascontiguousarray(W1)
        wm[f"b1_{l}"] = np.ascontiguousarray(b1.reshape(FH, 1))
        wm[f"w2_{l}"] = np.ascontiguousarray(W2 * g[:, None])
        wm[f"b2_{l}"] = np.ascontiguousarray((b2 + bt @ W2).reshape(1, FH))
    bn_g = A(params["bn_g"]) * inv_std
    bn_b = A(params["bn_b"])
    fcW, fcb = A(params["fc_W"]), A(params["fc_b"])
    wm["fcw"] = np.ascontiguousarray(fcW * bn_g[:, None])
    wm["fcb"] = np.ascontiguousarray((fcb + bn_b @ fcW).reshape(1, LAT))
    dec = params["dec"]
    wm["d1w"] = np.ascontiguousarray(A(dec[0]["W"]))
    wm["d1b"] = np.ascontiguousarray(A(dec[0]["b"]).reshape(HD, 1))
    wm["d2w"] = np.ascontiguousarray(A(dec[1]["W"]))
    wm["d2b"] = np.ascontiguousarray(A(dec[1]["b"]).reshape(HD, 1))
    gs = np.tile(np.eye(CH, dtype=np.float32).reshape(1, CH * CH), (SPAD, 1))
    gs[NPG:SPAD] = 0.0   # zero rows for padded nodes in lo tile
    wm["gsel"] = np.ascontiguousarray(gs)
    wm["onesr"] = np.ones((1, 128), dtype=np.float32)
    wm["zeros"] = np.zeros((1, CH * FH), dtype=np.float32)

    # padded final layer: column (i,j) = W3[:, ut_idx] (sym), 0 on diag
    W3, b3 = A(dec[2]["W"]), A(dec[2]["b"])
    iu0, iu1 = np.triu_indices(NPG, k=1)
    ut_of = np.zeros((NPG, NPG), dtype=np.int64)
    ut_of[iu0, iu1] = np.arange(N_UT)
    ut_of[iu1, iu0] = np.arange(N_UT)
    Wp = np.zeros((HD, NPG * NPG), dtype=np.float32)
    bp = np.full((NPG * NPG,), -10.0, dtype=np.float32)
    off = np.where(np.eye(NPG, dtype=bool).ravel(), -1, ut_of.ravel())
    nz = off >= 0
    Wp[:, nz] = W3[:, off[nz]]
    bp[nz] = b3[off[nz]]
    Wp = Wp.reshape(HD, NPG, NPG)
    bp = bp.reshape(NPG, NPG)

    in_maps = []
    for c in range(N_CORES):
        m = dict(wm)
        gs = c * GPC
        m["x"] = np.ascontiguousarray(x[gs * NPG:(gs + GPC) * NPG])
        m["at"] = np.ascontiguousarray(at[gs:gs + GPC])  # [g, s, d]
        r0 = c * ROWS_PC
        m["d3w"] = np.ascontiguousarray(
            Wp[:, r0:r0 + ROWS_PC, :].reshape(HD, NOUT))
        m["d3b"] = np.ascontiguousarray(
            bp[r0:r0 + ROWS_PC, :].reshape(1, NOUT))
        in_maps.append(m)
    return in_maps


def kernel(x, edge_index, batch, params):
    if "nc" not in _CACHE:
        _CACHE["nc"] = _build_program()
    nc = _CACHE["nc"]
    in_maps = _prep_host(x, edge_index, batch, params)

    from concourse.bass_utils import run_bass_kernel_spmd
    res = run_bass_kernel_spmd(nc, in_maps, list(range(N_CORES)))
    # core c holds adjacency rows [c*20, c*20+20) for all graphs
    slabs = [res.results[c]["out"].reshape(B, ROWS_PC, NPG)
             for c in range(N_CORES)]
    return np.ascontiguousarray(np.concatenate(slabs, axis=1))
